# revision 38
# baseline (speedup 1.0000x reference)
"""Trainium2 Bass kernel for the equivariant structure-denoising module.

Computation per node n:
    vec        = x[n, 1:4]                      # [3, 128]
    vec_local  = einsum('cf,ck->fk', vec, R)    # [128, 3]
    vec_norm   = sqrt(sum_c vec^2 + 1e-4)       # [128]
    a          = concat([x[n,0], vec_local.flat, vec_norm, extra[n]])   # [1152]
    h          = gelu(a @ W1 + b1)              # [1024], exact erf gelu
    y          = (h @ W2 + b2).reshape(4, 128)
    out        = concat([y[0:1], R @ y[1:4]])   # [4, 128]

Strategy (8 NeuronCores, data-parallel over nodes), bf16 end to end:
  - pad N 100000 -> 102400, shard 12800 nodes/core, 25 blocks of 512 nodes
  - all tensor data in bf16 (error gate is 2e-2; bf16 keeps us ~1e-3):
      * matmuls run 1 row/cycle (same as f32r) but transposes drop from
        1.5 to 1.0 cycles/row and every DVE op gets the 2x 16-bit mode
      * all DMA traffic halves
  - activations feature-major A^T [1152, 512] per block:
      * x0/extra chunks pre-transposed on host -> straight DMA into A^T
      * rotated vec chunks: DVE per-node rotations node-major (R entries
        are per-partition scalars), PE transposes 128x128 into bf16 PSUM,
        DVE drains (2x mode)
      * norm chunk: host also supplies the vec part feature-major (xvt);
        squares on ACT (Square lives in every ACT table), adds on GpSimd,
        one batched sqrt per block on ACT directly into A^T[4]
  - MLP1: lhsT = W1 tile (stationary), rhs = A^T chunk -> PSUM H^T chunk,
    GELU+bias on ScalarE PSUM->SBUF (bf16 out)
  - MLP2: lhsT = H^T chunk, rhs = W2 tile -> PSUM Y node-major; bias-add +
    output rotation on DVE (bf16), split DMA out (scalar part + vec part)
  - host converts inputs fp32->bf16 and output bf16->fp32 (layout only,
    no math beyond dtype rounding)
"""

import os
import sys

for _p in ("/opt/trn_rl_repo",):
    if _p not in sys.path and os.path.isdir(_p):
        sys.path.append(_p)

import ml_dtypes
import numpy as np

import concourse.bacc as bacc
import concourse.mybir as mybir
import concourse.tile as tile
from concourse.bass_utils import run_bass_kernel_spmd
from concourse.masks import make_identity

F32 = mybir.dt.float32
BF16 = mybir.dt.bfloat16
NP_BF16 = np.dtype(ml_dtypes.bfloat16)

N_FULL = 100_000
N_CORES = 8
FIBER = 128
EXTRA = 512
HIDDEN = 1024
IN_DIM = FIBER * 5 + EXTRA   # 1152
OUT_DIM = FIBER * 4          # 512
EPS = 1e-4

BLK = 512                    # nodes per block (PSUM bank = 512 fp32)
P = 128
NSUB = BLK // P              # 4 subtiles of 128 nodes
N_SHARD = 12_800             # nodes per core (25 blocks)
N_PAD = N_SHARD * N_CORES    # 102400
K_CH = IN_DIM // P           # 9 input chunks
H_CH = HIDDEN // P           # 8 hidden chunks

# k accumulation order inside each MLP1 psum: chunks that are plain DMAs
# first (x0, extra), then the on-chip-computed ones (vec_local, norm) so
# block 0 can start before the rotation pipeline has produced anything.
K_ORDER = [0, 5, 6, 7, 8, 1, 2, 3, 4]


def build_nc(nblk=N_SHARD // BLK):
    """Emit the per-core Bass program for nblk blocks of 512 nodes."""
    nshard = nblk * BLK
    nc = bacc.Bacc(None, target_bir_lowering=False)

    xv = nc.dram_tensor("xv", [nshard, 3 * FIBER], BF16, kind="ExternalInput")
    xvt = nc.dram_tensor("xvt", [3 * FIBER, nshard], BF16, kind="ExternalInput")
    x0t = nc.dram_tensor("x0t", [P, nshard], BF16, kind="ExternalInput")
    et = nc.dram_tensor("et", [EXTRA, nshard], BF16, kind="ExternalInput")
    rs = nc.dram_tensor("rs", [nblk * P, NSUB * 16], F32, kind="ExternalInput")
    w1 = nc.dram_tensor("w1", [IN_DIM, HIDDEN], BF16, kind="ExternalInput")
    w2 = nc.dram_tensor("w2", [HIDDEN, OUT_DIM], BF16, kind="ExternalInput")
    b1r = nc.dram_tensor("b1r", [P, H_CH], F32, kind="ExternalInput")
    b2b = nc.dram_tensor("b2b", [P, OUT_DIM], F32, kind="ExternalInput")
    outs = nc.dram_tensor("out", [nshard, 4 * FIBER], BF16, kind="ExternalOutput")

    mult = mybir.AluOpType.mult
    add = mybir.AluOpType.add
    Sqrt = mybir.ActivationFunctionType.Sqrt
    Square = mybir.ActivationFunctionType.Square
    Gelu = mybir.ActivationFunctionType.Gelu

    with tile.TileContext(nc) as tc:
        with (
            tc.tile_pool(name="consts", bufs=1) as consts,
            tc.tile_pool(name="at", bufs=3) as at_pool,
            tc.tile_pool(name="hsb", bufs=2) as h_pool,
            tc.tile_pool(name="xin", bufs=12) as x_pool,
            tc.tile_pool(name="vtt", bufs=3) as vt_pool,
            tc.tile_pool(name="sqp", bufs=3) as sq_pool,
            tc.tile_pool(name="rin", bufs=4) as r_pool,
            tc.tile_pool(name="vtmp", bufs=6) as v_pool,
            tc.tile_pool(name="vlp", bufs=12) as vl_pool,
            tc.tile_pool(name="ysb", bufs=4) as y_pool,
            tc.tile_pool(name="osb", bufs=4) as o_pool,
            tc.tile_pool(name="tpp", bufs=3, space="PSUM") as tp_psum,
            tc.tile_pool(name="hpp", bufs=3, space="PSUM") as h_psum,
            tc.tile_pool(name="ypp", bufs=2, space="PSUM") as y_psum,
        ):
            identity_f = consts.tile([P, P], F32)
            make_identity(nc, identity_f)
            identity = consts.tile([P, P], BF16)
            nc.vector.tensor_copy(identity, identity_f)
            eps_sb = consts.tile([P, 1], F32)
            nc.vector.memset(eps_sb, EPS)

            # W1 chunk DMAs split over two queues, K_ORDER-first so block 0
            # can begin accumulating as chunks land.
            w1_sb = [None] * K_CH
            for k in range(K_CH):
                w1_sb[k] = consts.tile([P, HIDDEN], BF16, name=f"w1_{k}",
                                       tag=f"w1_{k}")
            for qi, k in enumerate(K_ORDER):
                eng = nc.scalar if qi % 2 == 0 else nc.gpsimd
                eng.dma_start(out=w1_sb[k], in_=w1[k * P:(k + 1) * P, :])
            w2_sb = []
            for j in range(H_CH):
                t = consts.tile([P, OUT_DIM], BF16, name=f"w2_{j}", tag=f"w2_{j}")
                w2_sb.append(t)
            b1_sb = consts.tile([P, H_CH], F32)
            nc.gpsimd.dma_start(out=b1_sb, in_=b1r[:, :])
            b2_sb = consts.tile([P, OUT_DIM], F32)

            at_blocks = {}
            r_blocks = {}
            vls_blocks = {}
            nsq_blocks = {}

            def emit_prep_a(b):
                n0 = b * BLK
                at = [at_pool.tile([P, BLK], BF16, name=f"at_{k}", tag=f"at_{k}")
                      for k in range(K_CH)]
                at_blocks[b] = at
                r_sb = r_pool.tile([P, NSUB * 16], F32, name="r_sb", tag="r_sb")
                r_blocks[b] = r_sb
                nc.sync.dma_start(out=r_sb, in_=rs[b * P:(b + 1) * P, :])

                # x (node-major) first: it gates the longest path (rotations)
                x_sbs = []
                for i in range(NSUB):
                    base = n0 + i * P
                    x_sb = x_pool.tile([P, 3 * FIBER], BF16, name="x_sb", tag="x_sb")
                    nc.sync.dma_start(out=x_sb, in_=xv[base:base + P, :])
                    x_sbs.append(x_sb)

                # x0 and extra chunks: straight DMA from host-transposed DRAM
                nc.sync.dma_start(out=at[0], in_=x0t[:, n0:n0 + BLK])
                for t in range(4):
                    nc.sync.dma_start(
                        out=at[5 + t], in_=et[t * P:(t + 1) * P, n0:n0 + BLK])

                # feature-major vec for the norm path
                vts = []
                for c in range(3):
                    vt = vt_pool.tile([P, BLK], BF16, name=f"vt_{c}", tag=f"vt_{c}")
                    nc.scalar.dma_start(
                        out=vt, in_=xvt[c * P:(c + 1) * P, n0:n0 + BLK])
                    vts.append(vt)

                # per-node input rotations, node-major (scalars per partition)
                vls_all = []
                for i in range(NSUB):
                    x_sb = x_sbs[i]

                    def rsc(c, k):
                        col = i * 16 + c * 3 + k
                        return r_sb[:, col:col + 1]

                    v0 = x_sb[:, 0 * P:1 * P]
                    v1 = x_sb[:, 1 * P:2 * P]
                    v2 = x_sb[:, 2 * P:3 * P]

                    vls = []
                    for k in range(3):
                        ta = v_pool.tile([P, P], BF16, name="rot_a", tag="rot_a")
                        tb = v_pool.tile([P, P], BF16, name="rot_b", tag="rot_b")
                        vl = vl_pool.tile([P, P], BF16, name=f"vl_{k}", tag=f"vl_{k}")
                        # chain-first op on ACT (Copy+scale, in every table)
                        nc.scalar.mul(ta, v0, rsc(0, k))
                        nc.vector.scalar_tensor_tensor(
                            tb, v1, rsc(1, k), ta, op0=mult, op1=add)
                        nc.vector.scalar_tensor_tensor(
                            vl, v2, rsc(2, k), tb, op0=mult, op1=add)
                        vls.append(vl)
                    vls_all.append(vls)
                vls_blocks[b] = vls_all

                # norm chunk, feature-major: squares on ACT (Square is in
                # every ACT table), adds on GpSimd, batched sqrt on ACT
                sqs = []
                for c in range(3):
                    s = sq_pool.tile([P, BLK], BF16, name=f"s_{c}", tag=f"s_{c}")
                    nc.scalar.activation(s, vts[c], Square)
                    sqs.append(s)
                t01 = sq_pool.tile([P, BLK], BF16, name="t01", tag="t01")
                nc.gpsimd.tensor_add(t01, sqs[0], sqs[1])
                nsq = sq_pool.tile([P, BLK], BF16, name="nsq", tag="nsq")
                nc.gpsimd.tensor_add(nsq, t01, sqs[2])
                nc.scalar.activation(at[4], nsq, Sqrt, bias=eps_sb)

            def emit_prep_b(b):
                at = at_blocks[b]
                vls_all = vls_blocks.pop(b)
                # transpose vec_local tiles into A^T chunks 1..3 (bf16 PSUM,
                # 1 cycle/row), drain on DVE (2x 16-bit mode)
                for k in range(3):
                    pt = tp_psum.tile([P, BLK], BF16, name="tp", tag="tp")
                    for i in range(NSUB):
                        nc.tensor.transpose(
                            pt[:, i * P:(i + 1) * P], vls_all[i][k], identity)
                    nc.vector.tensor_copy(at[1 + k], pt)

            h_blocks = {}

            def emit_mlp1(b):
                at = at_blocks.pop(b)
                h_sb = []
                for j in range(H_CH):
                    hp = h_psum.tile([P, BLK], F32, name="hp", tag="hp")
                    for ki, k in enumerate(K_ORDER):
                        nc.tensor.matmul(
                            hp,
                            w1_sb[k][:, j * P:(j + 1) * P],
                            at[k],
                            start=(ki == 0), stop=(ki == K_CH - 1))
                    h = h_pool.tile([P, BLK], BF16, name=f"h_{j}", tag=f"h_{j}")
                    nc.scalar.activation(h, hp, Gelu, bias=b1_sb[:, j:j + 1])
                    h_sb.append(h)
                h_blocks[b] = h_sb

            def emit_mlp2(b):
                n0 = b * BLK
                r_sb = r_blocks.pop(b)
                h_sb = h_blocks.pop(b)

                def emit_rot_out(i, y_sb):
                    base = n0 + i * P

                    def rsc(c, k):
                        col = i * 16 + c * 3 + k
                        return r_sb[:, col:col + 1]

                    # scalar part goes straight out
                    nc.sync.dma_start(
                        out=outs[base:base + P, 0:P], in_=y_sb[:, 0:P])
                    o_sb = o_pool.tile([P, 3 * P], BF16, name="o_sb", tag="o_sb")
                    yv0 = y_sb[:, 1 * P:2 * P]
                    yv1 = y_sb[:, 2 * P:3 * P]
                    yv2 = y_sb[:, 3 * P:4 * P]
                    # vec_out_c = R[c,0]*yv0 + R[c,1]*yv1 + R[c,2]*yv2 (DVE)
                    for c in range(3):
                        ta = v_pool.tile([P, P], BF16, name="orot_a", tag="orot_a")
                        tb = v_pool.tile([P, P], BF16, name="orot_b", tag="orot_b")
                        if b == nblk - 1:
                            # tail: ACT is idle, shorten the DVE drain
                            nc.scalar.mul(ta, yv0, rsc(c, 0))
                        else:
                            nc.vector.tensor_scalar_mul(ta, yv0, rsc(c, 0))
                        nc.vector.scalar_tensor_tensor(
                            tb, yv1, rsc(c, 1), ta, op0=mult, op1=add)
                        nc.vector.scalar_tensor_tensor(
                            o_sb[:, c * P:(c + 1) * P], yv2, rsc(c, 2),
                            tb, op0=mult, op1=add)
                    nc.sync.dma_start(
                        out=outs[base:base + P, P:4 * P], in_=o_sb)

                # bias-add goes on DVE right after each subtile's matmuls so
                # the PSUM bank frees in time (2 banks); the 9-op rotation of
                # subtile i is deferred until after bias(i+1) is enqueued
                pend = []
                for i in range(NSUB):
                    yp = y_psum.tile([P, OUT_DIM], F32, name="yp", tag="yp")
                    for j in range(H_CH):
                        nc.tensor.matmul(
                            yp,
                            h_sb[j][:, i * P:(i + 1) * P],
                            w2_sb[j],
                            start=(j == 0), stop=(j == H_CH - 1))
                    y_sb = y_pool.tile([P, OUT_DIM], BF16, name="y_sb", tag="y_sb")
                    nc.vector.tensor_add(y_sb, yp, b2_sb)
                    if pend:
                        emit_rot_out(*pend.pop())
                    pend.append((i, y_sb))
                emit_rot_out(*pend.pop())

            # software pipeline; emission order = Tile priority / queue
            # order.  prep_a leads by TWO blocks so input DMAs are always
            # ahead of output DMAs on the sync queue — otherwise outs(b)
            # (which wait on the rotation chains) block inputs(b+2) and the
            # whole input pipeline runs one out-drain late.
            emit_prep_a(0)
            emit_prep_b(0)
            emit_prep_a(1)
            # W2/b2 first needed by MLP2 of block 0 (~25us in): defer their
            # DMAs behind block-0 input prep
            for j in range(H_CH):
                nc.gpsimd.dma_start(out=w2_sb[j], in_=w2[j * P:(j + 1) * P, :])
            nc.gpsimd.dma_start(out=b2_sb, in_=b2b[:, :])
            for b in range(nblk):
                if b + 2 < nblk:
                    emit_prep_a(b + 2)
                emit_mlp1(b)
                if b + 1 < nblk:
                    emit_prep_b(b + 1)
                emit_mlp2(b)

    nc.finalize()
    return nc


def prep_inputs(x, rotation_mats, extra_feats, W1, b1, W2, b2, nblk=N_SHARD // BLK):
    """Host-side shard + layout massaging. Returns per-core input maps."""
    nshard = nblk * BLK
    npad = nshard * N_CORES
    n = x.shape[0]

    x = np.asarray(x, dtype=np.float32)
    xv = np.ascontiguousarray(x[:, 1:4, :]).reshape(n, 3 * FIBER).astype(NP_BF16)
    x0 = np.ascontiguousarray(x[:, 0, :]).astype(NP_BF16)            # [n, 128]
    r = np.ascontiguousarray(np.asarray(rotation_mats, dtype=np.float32).reshape(n, 9))
    e = np.asarray(extra_feats, dtype=np.float32).astype(NP_BF16)
    if n < npad:
        pad = npad - n
        xv = np.concatenate([xv, np.zeros((pad, 3 * FIBER), NP_BF16)])
        x0 = np.concatenate([x0, np.zeros((pad, FIBER), NP_BF16)])
        r = np.concatenate([r, np.zeros((pad, 9), np.float32)])
        e = np.concatenate([e, np.zeros((pad, EXTRA), NP_BF16)])

    # W1 rows permuted: our A^T row order is [x0; vl_k k-major; vn; extra],
    # reference is [x0; vl (f,k) f-major; vn; extra]
    perm = np.arange(IN_DIM)
    for k in range(3):
        perm[P + k * P + np.arange(P)] = P + np.arange(P) * 3 + k
    w1p = np.ascontiguousarray(np.asarray(W1, dtype=np.float32)[perm, :]).astype(NP_BF16)
    w2 = np.ascontiguousarray(np.asarray(W2, dtype=np.float32)).astype(NP_BF16)
    b1r = np.ascontiguousarray(np.asarray(b1, dtype=np.float32).reshape(H_CH, P).T)
    b2b = np.ascontiguousarray(np.tile(np.asarray(b2, dtype=np.float32), (P, 1)))

    in_maps = []
    for c in range(N_CORES):
        sl = slice(c * nshard, (c + 1) * nshard)
        rc = r[sl].reshape(nblk, NSUB, P, 9).transpose(0, 2, 1, 3)  # [nblk,P,NSUB,9]
        rc16 = np.zeros((nblk, P, NSUB, 16), np.float32)
        rc16[..., :9] = rc
        in_maps.append({
            "xv": xv[sl],
            "xvt": np.ascontiguousarray(xv[sl].reshape(nshard, 3, FIBER)
                                        .transpose(1, 2, 0).reshape(3 * FIBER, nshard)),
            "x0t": np.ascontiguousarray(x0[sl].T),
            "et": np.ascontiguousarray(e[sl].T),
            "rs": rc16.reshape(nblk * P, NSUB * 16),
            "w1": w1p,
            "w2": w2,
            "b1r": b1r,
            "b2b": b2b,
        })
    return in_maps


_NC_CACHE = {}


def run(x, rotation_mats, extra_feats, W1, b1, W2, b2,
        nblk=N_SHARD // BLK, trace=False, use_f32r=True):
    key = (nblk,)
    if key not in _NC_CACHE:
        _NC_CACHE[key] = build_nc(nblk=nblk)
    nc = _NC_CACHE[key]
    in_maps = prep_inputs(x, rotation_mats, extra_feats, W1, b1, W2, b2, nblk=nblk)
    res = run_bass_kernel_spmd(nc, in_maps, list(range(N_CORES)), trace=trace)
    n = x.shape[0]
    full = np.concatenate([res.results[c]["out"] for c in range(N_CORES)], axis=0)
    out = full[:n].astype(np.float32).reshape(n, 4, FIBER)
    return out, res


def kernel(x, rotation_mats, extra_feats, W1, b1, W2, b2):
    out, _ = run(x, rotation_mats, extra_feats, W1, b1, W2, b2)
    return out


# revision 39
# speedup vs baseline: 1.0458x; 1.0458x over previous
"""Trainium2 Bass kernel for the equivariant structure-denoising module.

Computation per node n:
    vec        = x[n, 1:4]                      # [3, 128]
    vec_local  = einsum('cf,ck->fk', vec, R)    # [128, 3]
    vec_norm   = sqrt(sum_c vec^2 + 1e-4)       # [128]
    a          = concat([x[n,0], vec_local.flat, vec_norm, extra[n]])   # [1152]
    h          = gelu(a @ W1 + b1)              # [1024], exact erf gelu
    y          = (h @ W2 + b2).reshape(4, 128)
    out        = concat([y[0:1], R @ y[1:4]])   # [4, 128]

Strategy (8 NeuronCores, data-parallel over nodes), bf16 end to end:
  - pad N 100000 -> 102400, shard 12800 nodes/core, 25 blocks of 512 nodes
  - all tensor data in bf16 (error gate is 2e-2; bf16 keeps us ~1e-3):
      * matmuls run 1 row/cycle (same as f32r) but transposes drop from
        1.5 to 1.0 cycles/row and every DVE op gets the 2x 16-bit mode
      * all DMA traffic halves
  - activations feature-major A^T [1152, 512] per block:
      * x0/extra chunks pre-transposed on host -> straight DMA into A^T
      * rotated vec chunks: DVE per-node rotations node-major (R entries
        are per-partition scalars), PE transposes 128x128 into bf16 PSUM,
        DVE drains (2x mode)
      * norm chunk: host also supplies the vec part feature-major (xvt);
        squares on ACT (Square lives in every ACT table), adds on GpSimd,
        one batched sqrt per block on ACT directly into A^T[4]
  - MLP1: lhsT = W1 tile (stationary), rhs = A^T chunk -> PSUM H^T chunk,
    GELU+bias on ScalarE PSUM->SBUF (bf16 out)
  - MLP2: lhsT = H^T chunk, rhs = W2 tile -> PSUM Y node-major; bias-add +
    output rotation on DVE (bf16), split DMA out (scalar part + vec part)
  - host converts inputs fp32->bf16 and output bf16->fp32 (layout only,
    no math beyond dtype rounding)
"""

import os
import sys

for _p in ("/opt/trn_rl_repo",):
    if _p not in sys.path and os.path.isdir(_p):
        sys.path.append(_p)

import ml_dtypes
import numpy as np

import concourse.bacc as bacc
import concourse.mybir as mybir
import concourse.tile as tile
from concourse.bass_utils import run_bass_kernel_spmd
from concourse.masks import make_identity

F32 = mybir.dt.float32
BF16 = mybir.dt.bfloat16
NP_BF16 = np.dtype(ml_dtypes.bfloat16)

N_FULL = 100_000
N_CORES = 8
FIBER = 128
EXTRA = 512
HIDDEN = 1024
IN_DIM = FIBER * 5 + EXTRA   # 1152
OUT_DIM = FIBER * 4          # 512
EPS = 1e-4

BLK = 512                    # nodes per block (PSUM bank = 512 fp32)
P = 128
NSUB = BLK // P              # 4 subtiles of 128 nodes
N_SHARD = 12_800             # nodes per core (25 blocks)
N_PAD = N_SHARD * N_CORES    # 102400
K_CH = IN_DIM // P           # 9 input chunks
H_CH = HIDDEN // P           # 8 hidden chunks

# k accumulation order inside each MLP1 psum: chunks that are plain DMAs
# first (x0, extra), then the on-chip-computed ones (vec_local, norm) so
# block 0 can start before the rotation pipeline has produced anything.
K_ORDER = [0, 5, 6, 7, 8, 1, 2, 3, 4]


def build_nc(nblk=N_SHARD // BLK):
    """Emit the per-core Bass program for nblk blocks of 512 nodes."""
    nshard = nblk * BLK
    nc = bacc.Bacc(None, target_bir_lowering=False)

    xv = nc.dram_tensor("xv", [nshard, 3 * FIBER], BF16, kind="ExternalInput")
    xvt = nc.dram_tensor("xvt", [3 * FIBER, nshard], BF16, kind="ExternalInput")
    x0t = nc.dram_tensor("x0t", [P, nshard], BF16, kind="ExternalInput")
    et = nc.dram_tensor("et", [EXTRA, nshard], BF16, kind="ExternalInput")
    rs = nc.dram_tensor("rs", [nblk * P, NSUB * 16], F32, kind="ExternalInput")
    w1 = nc.dram_tensor("w1", [IN_DIM, HIDDEN], BF16, kind="ExternalInput")
    w2 = nc.dram_tensor("w2", [HIDDEN, OUT_DIM], BF16, kind="ExternalInput")
    b1r = nc.dram_tensor("b1r", [P, H_CH], F32, kind="ExternalInput")
    b2b = nc.dram_tensor("b2b", [P, OUT_DIM], F32, kind="ExternalInput")
    outs = nc.dram_tensor("out", [nshard, 4 * FIBER], BF16, kind="ExternalOutput")

    mult = mybir.AluOpType.mult
    add = mybir.AluOpType.add
    Sqrt = mybir.ActivationFunctionType.Sqrt
    Square = mybir.ActivationFunctionType.Square
    Gelu = mybir.ActivationFunctionType.Gelu

    with tile.TileContext(nc) as tc:
        with (
            tc.tile_pool(name="consts", bufs=1) as consts,
            tc.tile_pool(name="at", bufs=2) as at_pool,
            tc.tile_pool(name="hsb", bufs=2) as h_pool,
            tc.tile_pool(name="xin", bufs=8) as x_pool,
            tc.tile_pool(name="vtt", bufs=2) as vt_pool,
            tc.tile_pool(name="sqp", bufs=2) as sq_pool,
            tc.tile_pool(name="rin", bufs=3) as r_pool,
            tc.tile_pool(name="vtmp", bufs=4) as v_pool,
            tc.tile_pool(name="vlp", bufs=8) as vl_pool,
            tc.tile_pool(name="ysb", bufs=4) as y_pool,
            tc.tile_pool(name="osb", bufs=4) as o_pool,
            tc.tile_pool(name="tpp", bufs=3, space="PSUM") as tp_psum,
            tc.tile_pool(name="hpp", bufs=3, space="PSUM") as h_psum,
            tc.tile_pool(name="ypp", bufs=2, space="PSUM") as y_psum,
        ):
            identity_f = consts.tile([P, P], F32)
            make_identity(nc, identity_f)
            identity = consts.tile([P, P], BF16)
            nc.vector.tensor_copy(identity, identity_f)
            eps_sb = consts.tile([P, 1], F32)
            nc.vector.memset(eps_sb, EPS)

            # W1 chunk DMAs split over two queues, K_ORDER-first so block 0
            # can begin accumulating as chunks land.
            w1_sb = [None] * K_CH
            for k in range(K_CH):
                w1_sb[k] = consts.tile([P, HIDDEN], BF16, name=f"w1_{k}",
                                       tag=f"w1_{k}")
            for qi, k in enumerate(K_ORDER):
                eng = nc.scalar if qi % 2 == 0 else nc.gpsimd
                eng.dma_start(out=w1_sb[k], in_=w1[k * P:(k + 1) * P, :])
            w2_sb = []
            for j in range(H_CH):
                t = consts.tile([P, OUT_DIM], BF16, name=f"w2_{j}", tag=f"w2_{j}")
                w2_sb.append(t)
            b1_sb = consts.tile([P, H_CH], F32)
            nc.gpsimd.dma_start(out=b1_sb, in_=b1r[:, :])
            b2_sb = consts.tile([P, OUT_DIM], F32)

            at_blocks = {}
            r_blocks = {}
            vls_blocks = {}
            nsq_blocks = {}

            def emit_prep_a(b):
                n0 = b * BLK
                at = [at_pool.tile([P, BLK], BF16, name=f"at_{k}", tag=f"at_{k}")
                      for k in range(K_CH)]
                at_blocks[b] = at
                r_sb = r_pool.tile([P, NSUB * 16], F32, name="r_sb", tag="r_sb")
                r_blocks[b] = r_sb
                nc.sync.dma_start(out=r_sb, in_=rs[b * P:(b + 1) * P, :])

                # x (node-major) first: it gates the longest path (rotations)
                x_sbs = []
                for i in range(NSUB):
                    base = n0 + i * P
                    x_sb = x_pool.tile([P, 3 * FIBER], BF16, name="x_sb", tag="x_sb")
                    nc.sync.dma_start(out=x_sb, in_=xv[base:base + P, :])
                    x_sbs.append(x_sb)

                # x0 and extra chunks: straight DMA from host-transposed DRAM
                nc.sync.dma_start(out=at[0], in_=x0t[:, n0:n0 + BLK])
                for t in range(4):
                    nc.sync.dma_start(
                        out=at[5 + t], in_=et[t * P:(t + 1) * P, n0:n0 + BLK])

                # feature-major vec for the norm path
                vts = []
                for c in range(3):
                    vt = vt_pool.tile([P, BLK], BF16, name=f"vt_{c}", tag=f"vt_{c}")
                    nc.scalar.dma_start(
                        out=vt, in_=xvt[c * P:(c + 1) * P, n0:n0 + BLK])
                    vts.append(vt)

                # per-node input rotations, node-major (scalars per partition)
                vls_all = []
                for i in range(NSUB):
                    x_sb = x_sbs[i]

                    def rsc(c, k):
                        col = i * 16 + c * 3 + k
                        return r_sb[:, col:col + 1]

                    v0 = x_sb[:, 0 * P:1 * P]
                    v1 = x_sb[:, 1 * P:2 * P]
                    v2 = x_sb[:, 2 * P:3 * P]

                    vls = []
                    for k in range(3):
                        ta = v_pool.tile([P, P], BF16, name="rot_a", tag="rot_a")
                        tb = v_pool.tile([P, P], BF16, name="rot_b", tag="rot_b")
                        vl = vl_pool.tile([P, P], BF16, name=f"vl_{k}", tag=f"vl_{k}")
                        nc.vector.tensor_scalar_mul(ta, v0, rsc(0, k))
                        nc.vector.scalar_tensor_tensor(
                            tb, v1, rsc(1, k), ta, op0=mult, op1=add)
                        nc.vector.scalar_tensor_tensor(
                            vl, v2, rsc(2, k), tb, op0=mult, op1=add)
                        vls.append(vl)
                    vls_all.append(vls)
                vls_blocks[b] = vls_all

                # norm chunk, feature-major: squares on ACT (Square is in
                # every ACT table), adds on GpSimd, batched sqrt on ACT
                sqs = []
                for c in range(3):
                    s = sq_pool.tile([P, BLK], BF16, name=f"s_{c}", tag=f"s_{c}")
                    nc.scalar.activation(s, vts[c], Square)
                    sqs.append(s)
                t01 = sq_pool.tile([P, BLK], BF16, name="t01", tag="t01")
                nc.gpsimd.tensor_add(t01, sqs[0], sqs[1])
                nsq = sq_pool.tile([P, BLK], BF16, name="nsq", tag="nsq")
                nc.gpsimd.tensor_add(nsq, t01, sqs[2])
                nc.scalar.activation(at[4], nsq, Sqrt, bias=eps_sb)

            def emit_prep_b(b):
                at = at_blocks[b]
                vls_all = vls_blocks.pop(b)
                # transpose vec_local tiles into A^T chunks 1..3 (bf16 PSUM,
                # 1 cycle/row), drain on DVE (2x 16-bit mode)
                for k in range(3):
                    pt = tp_psum.tile([P, BLK], BF16, name="tp", tag="tp")
                    for i in range(NSUB):
                        nc.tensor.transpose(
                            pt[:, i * P:(i + 1) * P], vls_all[i][k], identity)
                    nc.scalar.copy(at[1 + k], pt)

            h_blocks = {}

            def emit_mlp1(b):
                at = at_blocks.pop(b)
                h_sb = []
                for j in range(H_CH):
                    hp = h_psum.tile([P, BLK], F32, name="hp", tag="hp")
                    for ki, k in enumerate(K_ORDER):
                        nc.tensor.matmul(
                            hp,
                            w1_sb[k][:, j * P:(j + 1) * P],
                            at[k],
                            start=(ki == 0), stop=(ki == K_CH - 1))
                    h = h_pool.tile([P, BLK], BF16, name=f"h_{j}", tag=f"h_{j}")
                    nc.scalar.activation(h, hp, Gelu, bias=b1_sb[:, j:j + 1])
                    h_sb.append(h)
                h_blocks[b] = h_sb

            def emit_mlp2(b):
                n0 = b * BLK
                r_sb = r_blocks.pop(b)
                h_sb = h_blocks.pop(b)

                def emit_rot_out(i, y_sb):
                    base = n0 + i * P

                    def rsc(c, k):
                        col = i * 16 + c * 3 + k
                        return r_sb[:, col:col + 1]

                    # scalar part goes straight out
                    nc.sync.dma_start(
                        out=outs[base:base + P, 0:P], in_=y_sb[:, 0:P])
                    o_sb = o_pool.tile([P, 3 * P], BF16, name="o_sb", tag="o_sb")
                    yv0 = y_sb[:, 1 * P:2 * P]
                    yv1 = y_sb[:, 2 * P:3 * P]
                    yv2 = y_sb[:, 3 * P:4 * P]
                    # vec_out_c = R[c,0]*yv0 + R[c,1]*yv1 + R[c,2]*yv2 (DVE)
                    for c in range(3):
                        ta = v_pool.tile([P, P], BF16, name="orot_a", tag="orot_a")
                        tb = v_pool.tile([P, P], BF16, name="orot_b", tag="orot_b")
                        if b == nblk - 1:
                            nc.scalar.mul(ta, yv0, rsc(c, 0))
                        else:
                            nc.vector.tensor_scalar_mul(ta, yv0, rsc(c, 0))
                        nc.vector.scalar_tensor_tensor(
                            tb, yv1, rsc(c, 1), ta, op0=mult, op1=add)
                        nc.vector.scalar_tensor_tensor(
                            o_sb[:, c * P:(c + 1) * P], yv2, rsc(c, 2),
                            tb, op0=mult, op1=add)
                    nc.sync.dma_start(
                        out=outs[base:base + P, P:4 * P], in_=o_sb)

                # bias-add goes on DVE right after each subtile's matmuls so
                # the PSUM bank frees in time (2 banks); the 9-op rotation of
                # subtile i is deferred until after bias(i+1) is enqueued
                pend = []
                for i in range(NSUB):
                    yp = y_psum.tile([P, OUT_DIM], F32, name="yp", tag="yp")
                    for j in range(H_CH):
                        nc.tensor.matmul(
                            yp,
                            h_sb[j][:, i * P:(i + 1) * P],
                            w2_sb[j],
                            start=(j == 0), stop=(j == H_CH - 1))
                    y_sb = y_pool.tile([P, OUT_DIM], BF16, name="y_sb", tag="y_sb")
                    nc.vector.tensor_add(y_sb, yp, b2_sb)
                    if pend:
                        emit_rot_out(*pend.pop())
                    pend.append((i, y_sb))
                emit_rot_out(*pend.pop())

            # software pipeline; emission order = Tile priority / queue order
            emit_prep_a(0)
            emit_prep_b(0)
            # W2/b2 first needed by MLP2 of block 0 (~25us in): defer their
            # DMAs behind block-0 input prep
            for j in range(H_CH):
                nc.gpsimd.dma_start(out=w2_sb[j], in_=w2[j * P:(j + 1) * P, :])
            nc.gpsimd.dma_start(out=b2_sb, in_=b2b[:, :])
            for b in range(nblk):
                if b + 1 < nblk:
                    emit_prep_a(b + 1)
                emit_mlp1(b)
                if b + 1 < nblk:
                    emit_prep_b(b + 1)
                emit_mlp2(b)

    nc.finalize()
    return nc


def prep_inputs(x, rotation_mats, extra_feats, W1, b1, W2, b2, nblk=N_SHARD // BLK):
    """Host-side shard + layout massaging. Returns per-core input maps."""
    nshard = nblk * BLK
    npad = nshard * N_CORES
    n = x.shape[0]

    x = np.asarray(x, dtype=np.float32)
    xv = np.ascontiguousarray(x[:, 1:4, :]).reshape(n, 3 * FIBER).astype(NP_BF16)
    x0 = np.ascontiguousarray(x[:, 0, :]).astype(NP_BF16)            # [n, 128]
    r = np.ascontiguousarray(np.asarray(rotation_mats, dtype=np.float32).reshape(n, 9))
    e = np.asarray(extra_feats, dtype=np.float32).astype(NP_BF16)
    if n < npad:
        pad = npad - n
        xv = np.concatenate([xv, np.zeros((pad, 3 * FIBER), NP_BF16)])
        x0 = np.concatenate([x0, np.zeros((pad, FIBER), NP_BF16)])
        r = np.concatenate([r, np.zeros((pad, 9), np.float32)])
        e = np.concatenate([e, np.zeros((pad, EXTRA), NP_BF16)])

    # W1 rows permuted: our A^T row order is [x0; vl_k k-major; vn; extra],
    # reference is [x0; vl (f,k) f-major; vn; extra]
    perm = np.arange(IN_DIM)
    for k in range(3):
        perm[P + k * P + np.arange(P)] = P + np.arange(P) * 3 + k
    w1p = np.ascontiguousarray(np.asarray(W1, dtype=np.float32)[perm, :]).astype(NP_BF16)
    w2 = np.ascontiguousarray(np.asarray(W2, dtype=np.float32)).astype(NP_BF16)
    b1r = np.ascontiguousarray(np.asarray(b1, dtype=np.float32).reshape(H_CH, P).T)
    b2b = np.ascontiguousarray(np.tile(np.asarray(b2, dtype=np.float32), (P, 1)))

    in_maps = []
    for c in range(N_CORES):
        sl = slice(c * nshard, (c + 1) * nshard)
        rc = r[sl].reshape(nblk, NSUB, P, 9).transpose(0, 2, 1, 3)  # [nblk,P,NSUB,9]
        rc16 = np.zeros((nblk, P, NSUB, 16), np.float32)
        rc16[..., :9] = rc
        in_maps.append({
            "xv": xv[sl],
            "xvt": np.ascontiguousarray(xv[sl].reshape(nshard, 3, FIBER)
                                        .transpose(1, 2, 0).reshape(3 * FIBER, nshard)),
            "x0t": np.ascontiguousarray(x0[sl].T),
            "et": np.ascontiguousarray(e[sl].T),
            "rs": rc16.reshape(nblk * P, NSUB * 16),
            "w1": w1p,
            "w2": w2,
            "b1r": b1r,
            "b2b": b2b,
        })
    return in_maps


_NC_CACHE = {}


def run(x, rotation_mats, extra_feats, W1, b1, W2, b2,
        nblk=N_SHARD // BLK, trace=False, use_f32r=True):
    key = (nblk,)
    if key not in _NC_CACHE:
        _NC_CACHE[key] = build_nc(nblk=nblk)
    nc = _NC_CACHE[key]
    in_maps = prep_inputs(x, rotation_mats, extra_feats, W1, b1, W2, b2, nblk=nblk)
    res = run_bass_kernel_spmd(nc, in_maps, list(range(N_CORES)), trace=trace)
    n = x.shape[0]
    full = np.concatenate([res.results[c]["out"] for c in range(N_CORES)], axis=0)
    out = full[:n].astype(np.float32).reshape(n, 4, FIBER)
    return out, res


def kernel(x, rotation_mats, extra_feats, W1, b1, W2, b2):
    out, _ = run(x, rotation_mats, extra_feats, W1, b1, W2, b2)
    return out


# revision 40
# speedup vs baseline: 1.0753x; 1.0282x over previous
"""Trainium2 Bass kernel for the equivariant structure-denoising module.

Computation per node n:
    vec        = x[n, 1:4]                      # [3, 128]
    vec_local  = einsum('cf,ck->fk', vec, R)    # [128, 3]
    vec_norm   = sqrt(sum_c vec^2 + 1e-4)       # [128]
    a          = concat([x[n,0], vec_local.flat, vec_norm, extra[n]])   # [1152]
    h          = gelu(a @ W1 + b1)              # [1024], exact erf gelu
    y          = (h @ W2 + b2).reshape(4, 128)
    out        = concat([y[0:1], R @ y[1:4]])   # [4, 128]

Strategy (8 NeuronCores, data-parallel over nodes), bf16 end to end:
  - pad N 100000 -> 102400, shard 12800 nodes/core, 25 blocks of 512 nodes
  - all tensor data in bf16 (error gate is 2e-2; bf16 keeps us ~1e-3):
      * matmuls run 1 row/cycle (same as f32r) but transposes drop from
        1.5 to 1.0 cycles/row and every DVE op gets the 2x 16-bit mode
      * all DMA traffic halves
  - activations feature-major A^T [1152, 512] per block:
      * x0/extra chunks pre-transposed on host -> straight DMA into A^T
      * rotated vec chunks: DVE per-node rotations node-major (R entries
        are per-partition scalars), PE transposes 128x128 into bf16 PSUM,
        DVE drains (2x mode)
      * norm chunk: host also supplies the vec part feature-major (xvt);
        squares on ACT (Square lives in every ACT table), adds on GpSimd,
        one batched sqrt per block on ACT directly into A^T[4]
  - MLP1: lhsT = W1 tile (stationary), rhs = A^T chunk -> PSUM H^T chunk,
    GELU+bias on ScalarE PSUM->SBUF (bf16 out)
  - MLP2: lhsT = H^T chunk, rhs = W2 tile -> PSUM Y node-major; bias-add +
    output rotation on DVE (bf16), split DMA out (scalar part + vec part)
  - host converts inputs fp32->bf16 and output bf16->fp32 (layout only,
    no math beyond dtype rounding)
"""

import os
import sys

for _p in ("/opt/trn_rl_repo",):
    if _p not in sys.path and os.path.isdir(_p):
        sys.path.append(_p)

import ml_dtypes
import numpy as np

import concourse.bacc as bacc
import concourse.mybir as mybir
import concourse.tile as tile
from concourse.bass_utils import run_bass_kernel_spmd
from concourse.masks import make_identity

F32 = mybir.dt.float32
BF16 = mybir.dt.bfloat16
NP_BF16 = np.dtype(ml_dtypes.bfloat16)

N_FULL = 100_000
N_CORES = 8
FIBER = 128
EXTRA = 512
HIDDEN = 1024
IN_DIM = FIBER * 5 + EXTRA   # 1152
OUT_DIM = FIBER * 4          # 512
EPS = 1e-4

BLK = 512                    # nodes per block (PSUM bank = 512 fp32)
P = 128
NSUB = BLK // P              # 4 subtiles of 128 nodes
N_SHARD = 12_800             # nodes per core (25 blocks)
N_PAD = N_SHARD * N_CORES    # 102400
K_CH = IN_DIM // P           # 9 input chunks
H_CH = HIDDEN // P           # 8 hidden chunks

# k accumulation order inside each MLP1 psum: chunks that are plain DMAs
# first (x0, extra), then the on-chip-computed ones (vec_local, norm) so
# block 0 can start before the rotation pipeline has produced anything.
K_ORDER = [0, 5, 6, 7, 8, 1, 2, 3, 4]


def build_nc(nblk=N_SHARD // BLK):
    """Emit the per-core Bass program for nblk blocks of 512 nodes."""
    nshard = nblk * BLK
    nc = bacc.Bacc(None, target_bir_lowering=False)

    xv = nc.dram_tensor("xv", [nshard, 3 * FIBER], BF16, kind="ExternalInput")
    xvt = nc.dram_tensor("xvt", [3 * FIBER, nshard], BF16, kind="ExternalInput")
    x0t = nc.dram_tensor("x0t", [P, nshard], BF16, kind="ExternalInput")
    et = nc.dram_tensor("et", [EXTRA, nshard], BF16, kind="ExternalInput")
    rs = nc.dram_tensor("rs", [nblk * P, NSUB * 16], F32, kind="ExternalInput")
    w1 = nc.dram_tensor("w1", [IN_DIM, HIDDEN], BF16, kind="ExternalInput")
    w2 = nc.dram_tensor("w2", [HIDDEN, OUT_DIM], BF16, kind="ExternalInput")
    b1r = nc.dram_tensor("b1r", [P, H_CH], F32, kind="ExternalInput")
    b2b = nc.dram_tensor("b2b", [P, OUT_DIM], F32, kind="ExternalInput")
    outs = nc.dram_tensor("out", [nshard, 4 * FIBER], BF16, kind="ExternalOutput")

    mult = mybir.AluOpType.mult
    add = mybir.AluOpType.add
    Sqrt = mybir.ActivationFunctionType.Sqrt
    Square = mybir.ActivationFunctionType.Square
    Gelu = mybir.ActivationFunctionType.Gelu

    with tile.TileContext(nc) as tc:
        with (
            tc.tile_pool(name="consts", bufs=1) as consts,
            tc.tile_pool(name="at", bufs=2) as at_pool,
            tc.tile_pool(name="hsb", bufs=2) as h_pool,
            tc.tile_pool(name="xin", bufs=8) as x_pool,
            tc.tile_pool(name="vtt", bufs=2) as vt_pool,
            tc.tile_pool(name="sqp", bufs=2) as sq_pool,
            tc.tile_pool(name="rin", bufs=3) as r_pool,
            tc.tile_pool(name="vtmp", bufs=4) as v_pool,
            tc.tile_pool(name="vlp", bufs=8) as vl_pool,
            tc.tile_pool(name="ysb", bufs=4) as y_pool,
            tc.tile_pool(name="osb", bufs=4) as o_pool,
            tc.tile_pool(name="tpp", bufs=3, space="PSUM") as tp_psum,
            tc.tile_pool(name="hpp", bufs=3, space="PSUM") as h_psum,
            tc.tile_pool(name="ypp", bufs=2, space="PSUM") as y_psum,
        ):
            identity_f = consts.tile([P, P], F32)
            make_identity(nc, identity_f)
            identity = consts.tile([P, P], BF16)
            nc.vector.tensor_copy(identity, identity_f)
            eps_sb = consts.tile([P, 1], F32)
            nc.vector.memset(eps_sb, EPS)

            # W1 chunk DMAs split over two queues, K_ORDER-first so block 0
            # can begin accumulating as chunks land.
            w1_sb = [None] * K_CH
            for k in range(K_CH):
                w1_sb[k] = consts.tile([P, HIDDEN], BF16, name=f"w1_{k}",
                                       tag=f"w1_{k}")
            for qi, k in enumerate(K_ORDER):
                eng = nc.scalar if qi % 2 == 0 else nc.gpsimd
                eng.dma_start(out=w1_sb[k], in_=w1[k * P:(k + 1) * P, :])
            w2_sb = []
            for j in range(H_CH):
                t = consts.tile([P, OUT_DIM], BF16, name=f"w2_{j}", tag=f"w2_{j}")
                w2_sb.append(t)
            b1_sb = consts.tile([P, H_CH], F32)
            nc.gpsimd.dma_start(out=b1_sb, in_=b1r[:, :])
            b2_sb = consts.tile([P, OUT_DIM], F32)

            at_blocks = {}
            r_blocks = {}
            vls_blocks = {}
            nsq_blocks = {}

            def emit_prep_a(b):
                n0 = b * BLK
                at = [at_pool.tile([P, BLK], BF16, name=f"at_{k}", tag=f"at_{k}")
                      for k in range(K_CH)]
                at_blocks[b] = at
                r_sb = r_pool.tile([P, NSUB * 16], F32, name="r_sb", tag="r_sb")
                r_blocks[b] = r_sb
                nc.sync.dma_start(out=r_sb, in_=rs[b * P:(b + 1) * P, :])

                # x (node-major) first: it gates the longest path (rotations)
                x_sbs = []
                for i in range(NSUB):
                    base = n0 + i * P
                    x_sb = x_pool.tile([P, 3 * FIBER], BF16, name="x_sb", tag="x_sb")
                    nc.sync.dma_start(out=x_sb, in_=xv[base:base + P, :])
                    x_sbs.append(x_sb)

                # x0 and extra chunks: straight DMA from host-transposed DRAM
                nc.sync.dma_start(out=at[0], in_=x0t[:, n0:n0 + BLK])
                for t in range(4):
                    nc.sync.dma_start(
                        out=at[5 + t], in_=et[t * P:(t + 1) * P, n0:n0 + BLK])

                # feature-major vec for the norm path
                vts = []
                for c in range(3):
                    vt = vt_pool.tile([P, BLK], BF16, name=f"vt_{c}", tag=f"vt_{c}")
                    nc.sync.dma_start(
                        out=vt, in_=xvt[c * P:(c + 1) * P, n0:n0 + BLK])
                    vts.append(vt)

                # per-node input rotations, node-major (scalars per partition)
                vls_all = []
                for i in range(NSUB):
                    x_sb = x_sbs[i]

                    def rsc(c, k):
                        col = i * 16 + c * 3 + k
                        return r_sb[:, col:col + 1]

                    v0 = x_sb[:, 0 * P:1 * P]
                    v1 = x_sb[:, 1 * P:2 * P]
                    v2 = x_sb[:, 2 * P:3 * P]

                    vls = []
                    for k in range(3):
                        ta = v_pool.tile([P, P], BF16, name="rot_a", tag="rot_a")
                        tb = v_pool.tile([P, P], BF16, name="rot_b", tag="rot_b")
                        vl = vl_pool.tile([P, P], BF16, name=f"vl_{k}", tag=f"vl_{k}")
                        nc.vector.tensor_scalar_mul(ta, v0, rsc(0, k))
                        nc.vector.scalar_tensor_tensor(
                            tb, v1, rsc(1, k), ta, op0=mult, op1=add)
                        nc.vector.scalar_tensor_tensor(
                            vl, v2, rsc(2, k), tb, op0=mult, op1=add)
                        vls.append(vl)
                    vls_all.append(vls)
                vls_blocks[b] = vls_all

                # norm chunk, feature-major: squares on ACT (Square is in
                # every ACT table), adds on GpSimd, batched sqrt on ACT
                sqs = []
                for c in range(3):
                    s = sq_pool.tile([P, BLK], BF16, name=f"s_{c}", tag=f"s_{c}")
                    nc.scalar.activation(s, vts[c], Square)
                    sqs.append(s)
                t01 = sq_pool.tile([P, BLK], BF16, name="t01", tag="t01")
                nc.gpsimd.tensor_add(t01, sqs[0], sqs[1])
                nsq = sq_pool.tile([P, BLK], BF16, name="nsq", tag="nsq")
                nc.gpsimd.tensor_add(nsq, t01, sqs[2])
                nc.scalar.activation(at[4], nsq, Sqrt, bias=eps_sb)

            def emit_prep_b(b):
                at = at_blocks[b]
                vls_all = vls_blocks.pop(b)
                # transpose vec_local tiles into A^T chunks 1..3 (bf16 PSUM,
                # 1 cycle/row), drain on DVE (2x 16-bit mode)
                for k in range(3):
                    pt = tp_psum.tile([P, BLK], BF16, name="tp", tag="tp")
                    for i in range(NSUB):
                        nc.tensor.transpose(
                            pt[:, i * P:(i + 1) * P], vls_all[i][k], identity)
                    nc.scalar.copy(at[1 + k], pt)

            h_blocks = {}

            def emit_mlp1(b):
                at = at_blocks.pop(b)
                h_sb = []
                for j in range(H_CH):
                    hp = h_psum.tile([P, BLK], F32, name="hp", tag="hp")
                    for ki, k in enumerate(K_ORDER):
                        nc.tensor.matmul(
                            hp,
                            w1_sb[k][:, j * P:(j + 1) * P],
                            at[k],
                            start=(ki == 0), stop=(ki == K_CH - 1))
                    h = h_pool.tile([P, BLK], BF16, name=f"h_{j}", tag=f"h_{j}")
                    nc.scalar.activation(h, hp, Gelu, bias=b1_sb[:, j:j + 1])
                    h_sb.append(h)
                h_blocks[b] = h_sb

            def emit_mlp2(b):
                n0 = b * BLK
                r_sb = r_blocks.pop(b)
                h_sb = h_blocks.pop(b)

                def emit_rot_out(i, y_sb):
                    base = n0 + i * P

                    def rsc(c, k):
                        col = i * 16 + c * 3 + k
                        return r_sb[:, col:col + 1]

                    # scalar part goes straight out
                    nc.sync.dma_start(
                        out=outs[base:base + P, 0:P], in_=y_sb[:, 0:P])
                    o_sb = o_pool.tile([P, 3 * P], BF16, name="o_sb", tag="o_sb")
                    yv0 = y_sb[:, 1 * P:2 * P]
                    yv1 = y_sb[:, 2 * P:3 * P]
                    yv2 = y_sb[:, 3 * P:4 * P]
                    # vec_out_c = R[c,0]*yv0 + R[c,1]*yv1 + R[c,2]*yv2 (DVE)
                    for c in range(3):
                        ta = v_pool.tile([P, P], BF16, name="orot_a", tag="orot_a")
                        tb = v_pool.tile([P, P], BF16, name="orot_b", tag="orot_b")
                        if b == nblk - 1:
                            nc.scalar.mul(ta, yv0, rsc(c, 0))
                        else:
                            nc.vector.tensor_scalar_mul(ta, yv0, rsc(c, 0))
                        nc.vector.scalar_tensor_tensor(
                            tb, yv1, rsc(c, 1), ta, op0=mult, op1=add)
                        nc.vector.scalar_tensor_tensor(
                            o_sb[:, c * P:(c + 1) * P], yv2, rsc(c, 2),
                            tb, op0=mult, op1=add)
                    nc.sync.dma_start(
                        out=outs[base:base + P, P:4 * P], in_=o_sb)

                # bias-add goes on DVE right after each subtile's matmuls so
                # the PSUM bank frees in time (2 banks); the 9-op rotation of
                # subtile i is deferred until after bias(i+1) is enqueued
                pend = []
                for i in range(NSUB):
                    yp = y_psum.tile([P, OUT_DIM], F32, name="yp", tag="yp")
                    for j in range(H_CH):
                        nc.tensor.matmul(
                            yp,
                            h_sb[j][:, i * P:(i + 1) * P],
                            w2_sb[j],
                            start=(j == 0), stop=(j == H_CH - 1))
                    y_sb = y_pool.tile([P, OUT_DIM], BF16, name="y_sb", tag="y_sb")
                    nc.vector.tensor_add(y_sb, yp, b2_sb)
                    if pend:
                        emit_rot_out(*pend.pop())
                    pend.append((i, y_sb))
                emit_rot_out(*pend.pop())

            # software pipeline; emission order = Tile priority / queue order
            emit_prep_a(0)
            emit_prep_b(0)
            # W2/b2 first needed by MLP2 of block 0 (~25us in): defer their
            # DMAs behind block-0 input prep
            for j in range(H_CH):
                nc.gpsimd.dma_start(out=w2_sb[j], in_=w2[j * P:(j + 1) * P, :])
            nc.gpsimd.dma_start(out=b2_sb, in_=b2b[:, :])
            for b in range(nblk):
                if b + 1 < nblk:
                    emit_prep_a(b + 1)
                emit_mlp1(b)
                if b + 1 < nblk:
                    emit_prep_b(b + 1)
                emit_mlp2(b)

    nc.finalize()
    return nc


def prep_inputs(x, rotation_mats, extra_feats, W1, b1, W2, b2, nblk=N_SHARD // BLK):
    """Host-side shard + layout massaging. Returns per-core input maps."""
    nshard = nblk * BLK
    npad = nshard * N_CORES
    n = x.shape[0]

    x = np.asarray(x, dtype=np.float32)
    xv = np.ascontiguousarray(x[:, 1:4, :]).reshape(n, 3 * FIBER).astype(NP_BF16)
    x0 = np.ascontiguousarray(x[:, 0, :]).astype(NP_BF16)            # [n, 128]
    r = np.ascontiguousarray(np.asarray(rotation_mats, dtype=np.float32).reshape(n, 9))
    e = np.asarray(extra_feats, dtype=np.float32).astype(NP_BF16)
    if n < npad:
        pad = npad - n
        xv = np.concatenate([xv, np.zeros((pad, 3 * FIBER), NP_BF16)])
        x0 = np.concatenate([x0, np.zeros((pad, FIBER), NP_BF16)])
        r = np.concatenate([r, np.zeros((pad, 9), np.float32)])
        e = np.concatenate([e, np.zeros((pad, EXTRA), NP_BF16)])

    # W1 rows permuted: our A^T row order is [x0; vl_k k-major; vn; extra],
    # reference is [x0; vl (f,k) f-major; vn; extra]
    perm = np.arange(IN_DIM)
    for k in range(3):
        perm[P + k * P + np.arange(P)] = P + np.arange(P) * 3 + k
    w1p = np.ascontiguousarray(np.asarray(W1, dtype=np.float32)[perm, :]).astype(NP_BF16)
    w2 = np.ascontiguousarray(np.asarray(W2, dtype=np.float32)).astype(NP_BF16)
    b1r = np.ascontiguousarray(np.asarray(b1, dtype=np.float32).reshape(H_CH, P).T)
    b2b = np.ascontiguousarray(np.tile(np.asarray(b2, dtype=np.float32), (P, 1)))

    in_maps = []
    for c in range(N_CORES):
        sl = slice(c * nshard, (c + 1) * nshard)
        rc = r[sl].reshape(nblk, NSUB, P, 9).transpose(0, 2, 1, 3)  # [nblk,P,NSUB,9]
        rc16 = np.zeros((nblk, P, NSUB, 16), np.float32)
        rc16[..., :9] = rc
        in_maps.append({
            "xv": xv[sl],
            "xvt": np.ascontiguousarray(xv[sl].reshape(nshard, 3, FIBER)
                                        .transpose(1, 2, 0).reshape(3 * FIBER, nshard)),
            "x0t": np.ascontiguousarray(x0[sl].T),
            "et": np.ascontiguousarray(e[sl].T),
            "rs": rc16.reshape(nblk * P, NSUB * 16),
            "w1": w1p,
            "w2": w2,
            "b1r": b1r,
            "b2b": b2b,
        })
    return in_maps


_NC_CACHE = {}


def run(x, rotation_mats, extra_feats, W1, b1, W2, b2,
        nblk=N_SHARD // BLK, trace=False, use_f32r=True):
    key = (nblk,)
    if key not in _NC_CACHE:
        _NC_CACHE[key] = build_nc(nblk=nblk)
    nc = _NC_CACHE[key]
    in_maps = prep_inputs(x, rotation_mats, extra_feats, W1, b1, W2, b2, nblk=nblk)
    res = run_bass_kernel_spmd(nc, in_maps, list(range(N_CORES)), trace=trace)
    n = x.shape[0]
    full = np.concatenate([res.results[c]["out"] for c in range(N_CORES)], axis=0)
    out = full[:n].astype(np.float32).reshape(n, 4, FIBER)
    return out, res


def kernel(x, rotation_mats, extra_feats, W1, b1, W2, b2):
    out, _ = run(x, rotation_mats, extra_feats, W1, b1, W2, b2)
    return out
